# revision 5
# baseline (speedup 1.0000x reference)
"""CholProd layer kernel for 8 Trainium2 NeuronCores.

Math per (k, m) pair (batch element):
  x = z[k, m, :, 0]                      # 528 values
  L = tril(reshape(concat([x[32:], x[::-1]]), [32, 32]))   # fill_triangular
  A = L with diagonal exponentiated
  C = A @ A.T + 1e-6 * I                 # -> z_out[k, m] (flattened, 1024)
  sldj_out = sldj_in + 32*log(2) + sum_i (33 - i) * L[i, i]

Sharding: pure batch parallelism.  64 k-slices are split 8-per-core; each
core processes 4096 independent batch elements.

On-core pipeline (per 128-batch tile):
  1. DMA x-tile [128b x 528] (batch on partitions, contiguous rows).
  2. Build xc [128 x 1024] = fill_triangular source (shifted copy +
     reversed copy), tril-mask via GPSIMD affine_select, extract the
     diagonal (stride-33 positions), do the sldj reduction, exp the
     diagonal in place (ScalarE).
  3. DVE stream (32x32-block) transpose -> xt[32g + j, 32i + b] so every
     batch's A^T lives in a [32 x 32] stride-32 column slice of a
     partition group.
  4. TensorE 32x32 array tiling (tile_position): 16 independent
     sub-arrays each compute one batch's C = A^T.T @ A^T directly from
     the strided slices.  PSUM bank per row-group.
  5. DVE scalar_tensor_tensor evacuates PSUM, fusing the +1e-6*I add.
  6. DMA out (128B runs, dst-contiguous per batch).
"""

import numpy as np

K, M, N = 64, 512, 32
D_Z = 528
NSQ = N * N  # 1024
N_CORES = 8
K_PER_CORE = K // N_CORES  # 8
B_PER_CORE = K_PER_CORE * M  # 4096
TILE_B = 128
N_TILES_FULL = B_PER_CORE // TILE_B  # 32
DIAG_EPS = 1e-6

_PROG_CACHE = {}


def build_program(n_tiles=N_TILES_FULL):
    """Build the Bass/Tile program for one core processing n_tiles*128 batches."""
    from contextlib import ExitStack

    import concourse.bacc as bacc
    import concourse.mybir as mybir
    import concourse.tile as tile

    f32 = mybir.dt.float32
    B = n_tiles * TILE_B

    nc = bacc.Bacc(
        "TRN2", target_bir_lowering=False, debug=False, num_devices=N_CORES
    )

    z_in = nc.dram_tensor("z_in", [B, D_Z], f32, kind="ExternalInput")
    sl_in = nc.dram_tensor("sl_in", [B, 1], f32, kind="ExternalInput")
    z_out = nc.dram_tensor("z_out", [B, NSQ], f32, kind="ExternalOutput")
    sl_out = nc.dram_tensor("sl_out", [B, 1], f32, kind="ExternalOutput")

    # Constants.
    # sldj weights: coefficient (33 - i) for diagonal element i.
    w_np = np.broadcast_to(
        (33.0 - np.arange(N, dtype=np.float32))[None, :], (128, N)
    ).copy()
    # eps * I tiled over the 4x4 grid of 32x32 blocks of a [128, 128] evac tile.
    eye_np = np.zeros((128, 128), np.float32)
    for a in range(4):
        for s in range(4):
            eye_np[a * 32 : (a + 1) * 32, s * 32 : (s + 1) * 32] = (
                np.eye(N, dtype=np.float32) * DIAG_EPS
            )
    w_dram = nc.inline_tensor(w_np, "w_const")
    eye_dram = nc.inline_tensor(eye_np, "eye_const")
    LOG2_N = float(N * np.log(2.0))

    mult = mybir.AluOpType.mult
    add = mybir.AluOpType.add
    is_ge = mybir.AluOpType.is_ge
    Exp = mybir.ActivationFunctionType.Exp

    with tile.TileContext(nc) as tc, ExitStack() as ctx:
        consts = ctx.enter_context(tc.tile_pool(name="consts", bufs=1))
        xpool = ctx.enter_context(tc.tile_pool(name="x", bufs=3))
        xcpool = ctx.enter_context(tc.tile_pool(name="xc", bufs=2))
        xmpool = ctx.enter_context(tc.tile_pool(name="xm", bufs=2))
        xtpool = ctx.enter_context(tc.tile_pool(name="xt", bufs=3))
        dpool = ctx.enter_context(tc.tile_pool(name="d", bufs=2))
        jpool = ctx.enter_context(tc.tile_pool(name="junk", bufs=2))
        spool = ctx.enter_context(tc.tile_pool(name="s", bufs=2))
        slpool = ctx.enter_context(tc.tile_pool(name="sl", bufs=2))
        obpool = ctx.enter_context(tc.tile_pool(name="ob", bufs=3))
        psum = ctx.enter_context(
            tc.tile_pool(name="psum", bufs=8, space="PSUM")
        )

        wt = consts.tile([128, N], f32)
        nc.sync.dma_start(wt[:], w_dram.ap())
        eyet = consts.tile([128, 128], f32)
        nc.sync.dma_start(eyet[:], eye_dram.ap())

        for t in range(n_tiles):
            x = xpool.tile([128, D_Z], f32)
            nc.sync.dma_start(x[:], z_in.ap()[t * 128 : (t + 1) * 128, :])

            # xc = concat([x[32:], x[::-1]]) per partition
            xc = xcpool.tile([128, NSQ], f32)
            nc.vector.tensor_copy(xc[:, 0 : D_Z - N], x[:, N:D_Z])
            nc.vector.tensor_copy(xc[:, D_Z - N : NSQ], x[:, ::-1])

            # tril mask: viewing free dim as (i, j), keep where i - j >= 0
            xm = xmpool.tile([128, NSQ], f32)
            nc.gpsimd.affine_select(
                xm[:],
                xc[:],
                pattern=[[1, N], [-1, N]],
                compare_op=is_ge,
                fill=0.0,
                base=0,
                channel_multiplier=0,
            )

            # diagonal (pre-exp) at positions 33*i
            d = dpool.tile([128, N], f32)
            nc.vector.tensor_copy(d[:], xm[:, ::33])

            # sldj = sl_in + 32*log(2) + sum_i (33 - i) * d[i]
            junk = jpool.tile([128, N], f32)
            s1 = spool.tile([128, 1], f32)
            # s1 = sum_i w_i * d_i  (InstTensorTensorReduce faults at runtime
            # under the PJRT path, so use TensorScalarPtr with accum instead)
            nc.vector.scalar_tensor_tensor(
                out=junk[:],
                in0=d[:],
                scalar=1.0,
                in1=wt[:],
                op0=mult,
                op1=mult,
                accum_out=s1[:],
            )
            sli = slpool.tile([128, 1], f32)
            nc.sync.dma_start(sli[:], sl_in.ap()[t * 128 : (t + 1) * 128, :])
            slo = slpool.tile([128, 1], f32)
            # slo = (s1 + 32*log 2) + sl_in
            nc.vector.scalar_tensor_tensor(
                out=slo[:], in0=s1[:], scalar=LOG2_N, in1=sli[:], op0=add, op1=add
            )
            nc.sync.dma_start(sl_out.ap()[t * 128 : (t + 1) * 128, :], slo[:])

            # exponentiate diagonal in place
            nc.scalar.activation(xm[:, ::33], xm[:, ::33], Exp)

            # 32x32 block transpose: xt[32g + j, 32i + b] = A_{g,b}[i, j]
            xt = xtpool.tile([128, NSQ], f32)
            nc.vector.transpose(xt[:], xm[:])

            # SYRK: one matmul per batch on a 32x32 PE sub-array.
            # 64-batch units (h2 = 0, 1); within a unit, batch (g, b32) with
            # b32 = 16*h2 + h, h = 4*h1 + chi.
            # Row-group g -> PSUM bank g; tile (g, chi) writes
            # pg[32*chi + i, 32*h1 + l].
            for h2 in range(2):
                pgs = [
                    psum.tile([128, 128], f32, tag="psum", name=f"pg{t}_{h2}_{g}")
                    for g in range(4)
                ]
                for h in range(16):
                    chi = h % 4
                    h1 = h // 4
                    b32 = 16 * h2 + h
                    for g in range(4):
                        op = xt[32 * g : 32 * g + 32, b32::N]
                        nc.tensor.matmul(
                            pgs[g][32 * chi : 32 * chi + 32, 32 * h1 : 32 * h1 + 32],
                            lhsT=op,
                            rhs=op,
                            start=True,
                            stop=True,
                            tile_position=(32 * g, 32 * chi),
                        )
                # evacuate + add eps*I
                ob = obpool.tile([128, 512], f32)
                for g in range(4):
                    nc.vector.scalar_tensor_tensor(
                        out=ob[:, 128 * g : 128 * g + 128],
                        in0=pgs[g][:],
                        scalar=1.0,
                        in1=eyet[:],
                        op0=mult,
                        op1=add,
                    )
                # output DMA: batch = 128t + 32g + 16*h2 + 4*h1 + chi
                # ob[32*chi + i, 128*g + 32*h1 + l] -> z_out[batch, 32*i + l]
                for g in range(4):
                    for chi in range(4):
                        b0 = 128 * t + 32 * g + 16 * h2 + chi
                        dst = (
                            z_out.ap()[b0 : b0 + 13 : 4, :]
                            .rearrange("h1 (i l) -> h1 i l", i=N, l=N)
                            .transpose([1, 0, 2])
                        )
                        src = ob[
                            32 * chi : 32 * chi + 32, 128 * g : 128 * g + 128
                        ].rearrange("i (h1 l) -> i h1 l", h1=4, l=N)
                        nc.sync.dma_start(dst, src)

    nc.compile()
    return nc


def _get_prog():
    if "full" not in _PROG_CACHE:
        _PROG_CACHE["full"] = build_program()
    return _PROG_CACHE["full"]


def kernel(z, sum_log_det_jacobians):
    from concourse.bass_utils import run_bass_kernel_spmd

    nc = _get_prog()
    z = np.ascontiguousarray(np.asarray(z), dtype=np.float32)
    sl = np.ascontiguousarray(np.asarray(sum_log_det_jacobians), dtype=np.float32)
    assert z.shape == (K, M, D_Z, 1)
    assert sl.shape == (K, M)

    in_maps = []
    for c in range(N_CORES):
        ks = slice(c * K_PER_CORE, (c + 1) * K_PER_CORE)
        in_maps.append(
            {
                "z_in": z[ks].reshape(B_PER_CORE, D_Z),
                "sl_in": sl[ks].reshape(B_PER_CORE, 1),
            }
        )
    res = run_bass_kernel_spmd(nc, in_maps, list(range(N_CORES))).results

    z_out = np.empty((K, M, NSQ, 1), np.float32)
    sl_out = np.empty((K, M), np.float32)
    for c in range(N_CORES):
        ks = slice(c * K_PER_CORE, (c + 1) * K_PER_CORE)
        z_out[ks] = res[c]["z_out"].reshape(K_PER_CORE, M, NSQ, 1)
        sl_out[ks] = res[c]["sl_out"].reshape(K_PER_CORE, M)
    return z_out, sl_out
